# revision 21
# baseline (speedup 1.0000x reference)
"""Trainium2 Bass kernel for nn_Erode (5x5 all-ones SE, zero padding).

For an all-ones 5x5 structuring element, kornia-style Erode reduces to a
5x5 sliding-window MIN over the zero-padded image.  The min is separable:
a 5-tap vertical pass then a 5-tap horizontal pass.

Key perf ideas vs the fp32 baseline:
 * fp16 everywhere on chip (tolerance is 2e-2; fp16 rounding is ~5e-4):
   halves DMA bytes AND enables the DVE's 2x_1p mode (2 elem/cycle/lane)
   for tensor_tensor - but only when every operand AP is 4B-aligned with
   inner step +-1.
 * All vertical-pass shifts are whole row-slots (even element offsets
   since lw is even) -> always 4B-aligned -> 2x.  The horizontal pass is
   decomposed as T1=min(V,V+2), T2=min(T1,T1+1), out=min(T2,V+4) so that
   only T2 has an odd (2-byte) shift; T2 runs on GPSIMD (parity-blind
   Q7 cores), overlapping the DVE.  All remaining DVE ops are 2x.
 * DMA moves to the two HWDGE rings (sync + scalar/ACT engines), keeping
   GPSIMD free for compute.

Distribution: pure data parallel.  B*C = 24 images of 512x512 are split
3-per-core across 8 NeuronCores.  Inside a core, partition p = 40*i + j
owns K=13 output rows of image i as 17 free-dim row-slots (2+2 halo),
host-pre-gathered so every DMA is a large contiguous-per-partition
transfer.  Columns are processed in chunks (small first chunk = short
ramp; small last chunk = short tail).
"""

import numpy as np

# ---- fixed problem geometry (hardcoded per harness contract) ----
B, C, H, W = 8, 3, 512, 512
N_CORES = 8
IMGS = (B * C) // N_CORES  # 3 images per core
K = 13                   # output rows per partition
SLOTS = K + 4            # row-slots incl. 2+2 halo
PPI = 40                 # partitions per image = ceil(512/13)
NP = 128                 # DMA/compute partition width (8 junk stripes padded)
NP_DATA = IMGS * PPI     # 120 partitions carry real data
PAD_H = 2 + H + 12       # 526: top pad + data + tail pad (covers slot overrun)
PAD_W = 2 + W + 2        # 516
# column chunks: small first (ramp), big middle, small last (tail).
# NOTE: offloading the pairwise row-min to a SWDGE accum_op DMA was tried
# and is impossible: walrus only accepts cce_op=add for DMACopy, not
# min/max.  All min work stays on the DVE (provably minimal 6 passes).
CHUNKS = [(0, 170), (170, 365), (365, 512)]
# final-op col piece bounds per chunk (stores start early; the last
# piece is small so the end-of-kernel store drain is short)
H_BOUNDS = [[0, 170], [0, 195], [0, 83, 147]]
LWS = [c1 - c0 + 4 for c0, c1 in CHUNKS]
# input slot-split per chunk across the two HWDGE rings; chunk 0's first
# piece is small so the first vertical op starts ASAP; later splits are
# chosen so the two rings carry equal bytes (each streams ~107 B/ns)
SPLITS = [6, 10, 9]

IN_ELEMS = NP * SLOTS * sum(LWS)
OUT_ELEMS = NP * K * W

_cached = {}


def _build_program():
    import concourse.mybir as mybir
    from concourse import bass, bacc
    from concourse.tile import TileContext

    f16 = mybir.dt.float16
    MIN = mybir.AluOpType.min

    nc = bacc.Bacc("TRN2", target_bir_lowering=False, debug=False,
                   num_devices=N_CORES)
    xs = nc.dram_tensor("xs", [IN_ELEMS], f16, kind="ExternalInput")
    ys = nc.dram_tensor("ys", [OUT_ELEMS], f16, kind="ExternalOutput")

    in_off = 0
    out_off = 0
    with TileContext(nc) as tc:
        with tc.tile_pool(name="work", bufs=1) as pool:
            # issue ALL input DMAs up front, each chunk split across both
            # HWDGE rings (sync + scalar), so input streams at ~2x the
            # single-ring rate and is never behind the DVE.
            xt = []
            for ch in range(len(CHUNKS)):
                lw = LWS[ch]
                X = pool.tile([NP, SLOTS, lw], f16, tag=f"X{ch}")
                sp = SPLITS[ch]
                for (s0, s1), eng in (((0, sp), nc.sync),
                                      ((sp, SLOTS), nc.scalar)):
                    src = bass.AP(
                        tensor=xs,
                        offset=in_off + s0 * lw,
                        ap=[[SLOTS * lw, NP], [lw, s1 - s0], [1, lw]],
                    )
                    eng.dma_start(out=X[:, s0:s1], in_=src)
                in_off += NP * SLOTS * lw
                xt.append(X)

            for ch, (c0, c1) in enumerate(CHUNKS):
                lw = LWS[ch]
                cw = c1 - c0
                X = xt[ch]

                # vertical 5-tap min along row-slots (even offsets: 2x).
                # P = min(X[s],X[s+1]); Q = min(P[s],X[s+4]);
                # V = min(Q[s],P[s+2]).  Chunk 0's P is split at the
                # input-piece boundary so it starts when the small first
                # DMA piece lands; later chunks' input is resident well
                # before the DVE reaches them.
                NP_SL = SLOTS - 2  # 15
                P = pool.tile([NP, NP_SL, lw], f16, tag=f"P{ch}")
                if ch == 0:
                    sb = SPLITS[ch] - 1
                    nc.vector.tensor_tensor(out=P[:, 0:sb], in0=X[:, 0:sb],
                                            in1=X[:, 1:sb + 1], op=MIN)
                    nc.vector.tensor_tensor(
                        out=P[:, sb:NP_SL], in0=X[:, sb:NP_SL],
                        in1=X[:, sb + 1:NP_SL + 1], op=MIN)
                else:
                    nc.vector.tensor_tensor(out=P, in0=X[:, 0:NP_SL],
                                            in1=X[:, 1:NP_SL + 1], op=MIN)
                Q = pool.tile([NP, K, lw], f16, tag=f"Q{ch}")
                nc.vector.tensor_tensor(out=Q, in0=P[:, 0:K],
                                        in1=X[:, 4:SLOTS], op=MIN)
                V = pool.tile([NP, K, lw], f16, tag=f"V{ch}")
                nc.vector.tensor_tensor(out=V, in0=Q,
                                        in1=P[:, 2:K + 2], op=MIN)

                # horizontal 5-tap min along cols.
                # T1 = min(V[c],V[c+2])        (even shift, DVE 2x)
                # T2 = min(T1[c],T1[c+1])      (odd shift: GPSIMD)
                # out = min(T2[c],V[c+4])      (even shift, DVE 2x)
                # T1/T2 tiles padded to even row stride (lw-2) so the final
                # op's T2 operand rows stay 4B-aligned.
                T1 = pool.tile([NP, K, lw - 2], f16, tag=f"T1{ch}")
                nc.vector.tensor_tensor(out=T1, in0=V[:, :, 0:lw - 2],
                                        in1=V[:, :, 2:lw], op=MIN)
                T2 = pool.tile([NP, K, lw - 2], f16, tag=f"T2{ch}")
                nc.vector.tensor_tensor(out=T2[:, :, 0:lw - 3],
                                        in0=T1[:, :, 0:lw - 3],
                                        in1=T1[:, :, 1:lw - 2], op=MIN)

                # final tap, split into col pieces so stores start early
                bounds = H_BOUNDS[ch]
                for t in range(len(bounds) - 1):
                    b0, b1 = bounds[t], bounds[t + 1]
                    pw = b1 - b0
                    Hm = pool.tile([NP, K, pw], f16, tag=f"V2{ch}_{t}")
                    nc.vector.tensor_tensor(
                        out=Hm, in0=T2[:, :, b0:b0 + pw],
                        in1=V[:, :, 4 + b0:4 + b1], op=MIN)
                    dst = bass.AP(
                        tensor=ys,
                        offset=out_off,
                        ap=[[K * pw, NP], [pw, K], [1, pw]],
                    )
                    (nc.sync if t % 2 == 0 else nc.scalar).dma_start(
                        out=dst, in_=Hm)
                    out_off += NP * K * pw
    nc.compile()
    return nc


def _get_program():
    if "nc" not in _cached:
        _cached["nc"] = _build_program()
    return _cached["nc"]


# stripe gather index: [PPI, SLOTS] padded-row index per (j, s)
_ROW_IDX = (K * np.arange(PPI)[:, None] + np.arange(SLOTS)[None, :])


def _stripe_core_input(x3: np.ndarray) -> np.ndarray:
    """[3,512,512] f16 -> host-striped flat input (chunk-blocked)."""
    xp = np.zeros((IMGS, PAD_H, PAD_W), np.float16)
    xp[:, 2:2 + H, 2:2 + W] = x3
    stripes = np.zeros((NP, SLOTS + 1, PAD_W), np.float16)
    idx = (K * np.arange(PPI)[:, None] + np.arange(SLOTS + 1)[None, :])
    stripes[:NP_DATA] = xp[:, idx, :].reshape(NP_DATA, SLOTS + 1, PAD_W)
    parts = [
        stripes[:, :SLOTS, c0:c0 + lw].reshape(-1)
        for (c0, _), lw in zip(CHUNKS, LWS)
    ]
    return np.concatenate(parts)


def _out_pieces():
    pieces = []
    for ch, (c0, c1) in enumerate(CHUNKS):
        bounds = H_BOUNDS[ch]
        for t in range(len(bounds) - 1):
            pieces.append((c0 + bounds[t], bounds[t + 1] - bounds[t]))
    return pieces


_PIECES = None


def _unstripe_core_output(flat: np.ndarray) -> np.ndarray:
    """piece-blocked f16 output -> [3,512,512] f32."""
    global _PIECES
    if _PIECES is None:
        _PIECES = _out_pieces()
    stripes = np.empty((NP_DATA, K, W), np.float16)
    off = 0
    for col0, pw in _PIECES:
        blk = flat[off:off + NP * K * pw].reshape(NP, K, pw)
        stripes[:, :, col0:col0 + pw] = blk[:NP_DATA]
        off += NP * K * pw
    ys = stripes.reshape(IMGS, PPI, K, W)
    out = np.empty((IMGS, H, W), np.float32)
    full = (PPI - 1) * K  # 507 rows from full partitions
    out[:, :full] = ys[:, :PPI - 1].reshape(IMGS, full, W)
    out[:, full:] = ys[:, PPI - 1, :H - full]
    return out


def _run_on_hw(x24: np.ndarray, trace: bool = False):
    from concourse.bass_utils import run_bass_kernel_spmd
    nc = _get_program()
    x24 = x24.astype(np.float16)
    in_maps = [
        {"xs": _stripe_core_input(x24[IMGS * k:IMGS * (k + 1)])}
        for k in range(N_CORES)
    ]
    try:
        res = run_bass_kernel_spmd(nc, in_maps, list(range(N_CORES)),
                                   trace=trace)
    except Exception:
        import time
        time.sleep(5)
        res = run_bass_kernel_spmd(nc, in_maps, list(range(N_CORES)),
                                   trace=trace)
    out = np.stack([
        _unstripe_core_output(res.results[k]["ys"]) for k in range(N_CORES)
    ])
    return out.reshape(B, C, H, W), res


def _erode_reference_np(x: np.ndarray, se: np.ndarray) -> np.ndarray:
    """Generic fallback faithful to the kornia-style formula (numpy)."""
    kh, kw = se.shape
    ph, pw = kh // 2, kw // 2
    xpad = np.pad(x, ((0, 0), (0, 0), (ph, ph), (pw, pw)))
    out = None
    for r in range(kh):
        for c in range(kw):
            shifted = xpad[:, :, r:r + x.shape[2], c:c + x.shape[3]]
            bias = se[r, c] - 1.0
            val = shifted - bias if bias >= 0.0 else np.full_like(shifted, -bias)
            out = val if out is None else np.minimum(out, val)
    return out.astype(x.dtype)


def kernel(x, se):
    x = np.asarray(x, dtype=np.float32)
    se = np.asarray(se, dtype=np.float32)
    if se.shape != (5, 5) or not np.all(se == 1.0) or x.shape != (B, C, H, W):
        return _erode_reference_np(x, se)
    x24 = np.ascontiguousarray(x.reshape(B * C, H, W))
    out, _ = _run_on_hw(x24, trace=False)
    return out
